# revision 1
# baseline (speedup 1.0000x reference)
"""Trainium2 Bass kernel for nn_NeuralNetworkSimplified (binarized 4-layer MLP + BN).

Math
----
reference computes, per hidden layer l (gamma=1, beta=0, biases b_l arbitrary):
    z = sign(a) @ sign(W).T + sign(b)
    h = clip(batchnorm_train(z), -1, 1)
and the next layer only consumes sign(h).  Since batchnorm's rsqrt(var+eps) > 0
and gamma=1/beta=0, sign(h) = sign(z - mean_batch(z)); the sign(b) bias shifts
z and its mean equally, so it cancels.  The network reduces to exact integer
arithmetic.  On device we use an asymmetric encoding:
    W~ = sign(W) in {-1, +1}   (fp8, exact)
    B  = 1{a >= 0} in {0, 1}   (fp8, exact)
    p[m,n] = sum_k W~[k,m] * B[k,n]        (psum, exact ints)
    true Z = 2p - rowsum(W~)[m]; the rowsum term is constant per feature m, so
    it cancels in the batch-mean comparison:
        sign(Z - colmean(Z)) = 1{p >= pmean},  pmean = (W~ @ u) / 16384,
    where u[k] = global colsum of B (AllReduce of per-core counts).
The threshold pmean is computed on the PE with a tiny 2-column stationary
[u - 8192 (fp16 exact, |.| <= 2048), 8192] against the already-resident W~
tiles (moving operand), using 32-aligned column groups of one PSUM bank --
near-zero weight-load cost.  The final layer outputs 2*p4 - rowsum(W4~) + sign(b4).

Sharding: batch 16384 -> 8 cores x 2048, feature-major activations; only the
tiny u vectors are AllReduced between layers.  Host prep is layout-only
(transpose + bf16 cast); all FLOPs including sign() run on device.
"""

import numpy as np
import ml_dtypes

B, D, H1, H2, H3, C = 16384, 3072, 2048, 2048, 1024, 512
NCORES = 8
BL = B // NCORES          # 2048 rows per core
NF = 512                  # batch free-dim chunk (one psum bank)
NCH = BL // NF            # 4 chunks
LAYERS = [(D, H1), (H1, H2), (H2, H3), (H3, C)]
UDIMS = [D, H1, H2]       # length of u vector feeding each hidden layer's mean

_CACHE = {}


def _build_module():
    import concourse.bass as bass
    import concourse.mybir as mybir
    import concourse.tile as tile
    from concourse import bacc

    mdt = mybir.dt
    FP8 = mdt.float8e4
    ALU = mybir.AluOpType
    ACTF = mybir.ActivationFunctionType
    DR = mybir.MatmulPerfMode.DoubleRow

    nc = bacc.Bacc(
        "TRN2",
        target_bir_lowering=False,
        debug=False,
        num_devices=NCORES,
    )

    xT = nc.dram_tensor("xT", [D, BL], mdt.bfloat16, kind="ExternalInput").ap()
    wT = [
        nc.dram_tensor(f"w{i + 1}t", [K, H], mdt.bfloat16, kind="ExternalInput").ap()
        for i, (K, H) in enumerate(LAYERS)
    ]
    b4 = nc.dram_tensor("b4", [C, 1], mdt.float32, kind="ExternalInput").ap()
    outT = nc.dram_tensor("outT", [C, BL], mdt.float32, kind="ExternalOutput").ap()

    cc_in = [
        [nc.dram_tensor(f"cc_in{i}_{h}", [128, U // 256], mdt.float32).ap()
         for h in range(2)]
        for i, U in enumerate(UDIMS)
    ]
    cc_out = [
        [nc.dram_tensor(f"cc_out{i}_{h}", [128, U // 256], mdt.float32,
                        addr_space="Shared").ap()
         for h in range(2)]
        for i, U in enumerate(UDIMS)
    ]
    # scratch for cross-partition shuffle of the threshold rows (t) per layer
    tscr = [
        nc.dram_tensor(f"tscr{i}", [2, H // 512, 512], mdt.float32).ap()
        for i, H in enumerate([H1, H2, H3])
    ]
    rwscr = nc.dram_tensor("rwscr", [1, C], mdt.float32).ap()
    ccw_in = nc.dram_tensor("ccw_in", [128, 1], mdt.float32).ap()
    ccw_out = nc.dram_tensor("ccw_out", [128, 1], mdt.float32, addr_space="Shared").ap()

    with tile.TileContext(nc, num_cores=NCORES) as tc:
        with (
            tc.tile_pool(name="raw", bufs=2) as raw,       # staging bf16 pair tiles
            tc.tile_pool(name="sA", bufs=12) as sA,        # B0, B2 pair tiles
            tc.tile_pool(name="sB", bufs=8) as sB,         # B1, B3 pair tiles
            tc.tile_pool(name="wA", bufs=12) as wA,        # W1~, W3~ pair tiles
            tc.tile_pool(name="wB", bufs=8) as wB,         # W2~, W4~ pair tiles
            tc.tile_pool(name="stat", bufs=1) as stat,     # u/t/bias vectors
            tc.tile_pool(name="zd", bufs=3) as zd,         # deferred m0 psum spill
            tc.tile_pool(name="ou", bufs=2) as ou,         # output staging
            tc.tile_pool(name="pz", bufs=8, space="PSUM") as pz,
        ):
            # ---- PE warm-up while the prologue runs on DMA/DVE ----
            warm = stat.tile([128, 128], FP8, tag="warm")
            nc.vector.memset(warm, 1.0)
            wps = pz.tile([128, 128], mdt.float32, tag="pz", name="warmps")
            for i in range(36):
                nc.tensor.matmul(wps, warm, warm, start=True, stop=True)

            # warm the collective stream early: the first CC op pays ~60-90us
            # of stream setup; do it on garbage concurrently with the prologue
            ccwt = stat.tile([128, 1], mdt.float32, tag="ccwt")
            nc.vector.memset(ccwt, 0.0)
            nc.gpsimd.dma_start(ccw_in, ccwt)
            nc.gpsimd.collective_compute(
                "AllReduce", ALU.add,
                replica_groups=[list(range(NCORES))],
                ins=[ccw_in], outs=[ccw_out],
            )

            # tiny positive bias for ACT Sign: breaks w==0 ties toward +1
            biap = stat.tile([128, 1], mdt.float32, tag="biap")
            nc.vector.memset(biap, 1e-30)

            # ---- sign(b4) as +-1 per-partition vector, [128, C//128] ----
            b4_sb = stat.tile([128, C // 128], mdt.float32, tag="rawb")
            nc.sync.dma_start(b4_sb, b4.rearrange("(o p) q -> p (o q)", p=128))
            sb4 = stat.tile([128, C // 128], mdt.float32, tag="sb4")
            nc.gpsimd.tensor_scalar(
                out=sb4, in0=b4_sb, scalar1=0.0, scalar2=2.0,
                op0=ALU.is_ge, op1=ALU.mult,
            )
            nc.gpsimd.tensor_scalar_add(sb4, sb4, -1.0)

            # ---- binarize helpers ----
            def prep_w_pair(pool, idx, kp, eng):
                """bf16 dram pair -> fp8 +-1 pair tile (sign, exact)."""
                K, H = LAYERS[idx]
                w8 = pool.tile([128, 2, H], FP8, tag="w", name=f"w{idx}_{kp}")
                for h in range(2):
                    rt = raw.tile([128, H], mdt.bfloat16, tag="raww",
                                  name=f"rw{idx}_{kp}_{h}")
                    nc.sync.dma_start(
                        rt,
                        wT[idx][(2 * kp + h) * 128:(2 * kp + h + 1) * 128, :],
                    )
                    if eng == "act":
                        nc.scalar.activation(w8[:, h, :], rt,
                                             ACTF.Sign, bias=biap, scale=1.0)
                    else:
                        e = nc.vector if eng == "dve" else nc.gpsimd
                        e.tensor_scalar(
                            out=w8[:, h, :], in0=rt, scalar1=0.0,
                            scalar2=2.0, op0=ALU.is_ge, op1=ALU.mult,
                        )
                        e.tensor_scalar_add(w8[:, h, :], w8[:, h, :], -1.0)
                return w8

            # ---- u AllReduce, half-granular: AR-a fires mid-layer so the
            # next layer's first t-matmuls never wait on the full reduction
            def allreduce_u_half(idx, u_sb, half):
                HC = UDIMS[idx] // 256
                sl = slice(half * HC, (half + 1) * HC)
                nc.gpsimd.dma_start(cc_in[idx][half], u_sb[:, sl])
                nc.gpsimd.collective_compute(
                    "AllReduce",
                    ALU.add,
                    replica_groups=[list(range(NCORES))],
                    ins=[cc_in[idx][half]],
                    outs=[cc_out[idx][half]],
                )
                ug = stat.tile([128, HC], mdt.float32, tag=f"ug{idx}_{half}")
                nc.gpsimd.dma_start(ug, cc_out[idx][half])
                # dig[:, c, 0] = u - 8192 (exact fp16), dig[:, c, 1] = 8192
                dig = stat.tile([128, HC, 2], mdt.float16, tag=f"dig{idx}_{half}")
                nc.vector.memset(dig[:, :, 1:2], 8192.0)
                nc.vector.tensor_scalar_add(dig[:, :, 0:1], ug.unsqueeze(2), -8192.0)
                return dig

            # ---- x prep: B0 = 1{x>=0} (fp8 {0,1}) + u0 counts fused ----
            B0 = []
            u0 = stat.tile([128, D // 128], mdt.float32, tag="u0")
            # W1 pair 0 first so L1-m0's first matmuls start ASAP
            W8_1 = [prep_w_pair(wA, 0, 0, "act")]
            for kp in range(D // 256):
                s8 = sA.tile([128, 2, BL], FP8, tag="s", name=f"s0_{kp}")
                for h in range(2):
                    rt = raw.tile([128, BL], mdt.bfloat16, tag="rawx",
                                  name=f"rx{kp}_{h}")
                    nc.sync.dma_start(
                        rt,
                        xT[(2 * kp + h) * 128:(2 * kp + h + 1) * 128, :],
                    )
                    nc.vector.tensor_scalar(
                        out=s8[:, h, :], in0=rt, scalar1=0.0,
                        scalar2=0.0, op0=ALU.is_ge, op1=ALU.add,
                        accum_out=u0[:, 2 * kp + h:2 * kp + h + 1],
                    )
                B0.append(s8)
                # x leads W1 2:1 so u0 (and its AllReduce) finish early;
                # W1's tail overlaps L1 m0's streamed matmuls
                if kp % 2 == 1 and kp // 2 + 1 < D // 512:
                    W8_1.append(prep_w_pair(wA, 0, kp // 2 + 1, "act"))
                if kp == D // 512 - 1:
                    dig0a = allreduce_u_half(0, u0, 0)
            dig0b = allreduce_u_half(0, u0, 1)
            for kp in range(D // 512, D // 256):
                W8_1.append(prep_w_pair(wA, 0, kp, "act"))

            W8_2 = [prep_w_pair(wB, 1, kp, "act")
                    for kp in range(H1 // 256)]

            # ---- one layer ----
            def layer(l, A8, W8, dig, S_out, u_out, halfar=None):
                K, H = LAYERS[l]
                KT, MT = K // 128, H // 128
                NS = H // 512          # 512-wide feature slices for the t rows
                t_sc = stat.tile([128, MT], mdt.float32, tag=f"tsc{l}",
                                 name=f"tsc{l}") if l < 3 else None
                up = stat.tile([128, MT, NCH], mdt.float32, tag=f"up{l}",
                               name=f"up{l}") if l < 3 else None
                NDEF = 3               # m0..m2 spilled to SBUF, m3 held in psum
                zdefs = [zd.tile([128, BL], mdt.float16, tag="zd",
                                 name=f"zd{l}_{m}") for m in range(NDEF)] \
                    if l < 3 else None
                held = None            # m==NDEF psums held until t_sc is ready
                pending = []           # spilled blocks awaiting their S-write

                def mains(m):
                    mc = slice(m * 128, (m + 1) * 128)
                    psums = [
                        pz.tile([128, NF], mdt.float32, tag="pz",
                                name=f"pz{l}_{m}_{n}")
                        for n in range(NCH)
                    ]
                    for kp in range(KT // 2):
                        st, sp = kp == 0, kp == KT // 2 - 1
                        wsl = W8[kp][:, :, mc]
                        for n in range(NCH):
                            nc.tensor.matmul(
                                psums[n], wsl, A8[kp][:, :, n * NF:(n + 1) * NF],
                                start=st, stop=sp, perf_mode=DR,
                            )
                    return psums

                def swrite(m, srcs):
                    # S_out = 1{p >= t}, u partials fused via accum_out
                    for n in range(NCH):
                        nc.vector.tensor_scalar(
                            out=S_out[m // 2][:, m % 2, n * NF:(n + 1) * NF],
                            in0=srcs[n], scalar1=t_sc[:, m:m + 1], scalar2=0.0,
                            op0=ALU.is_ge, op1=ALU.add,
                            accum_out=up[:, m, n:n + 1],
                        )
                    if u_out is not None:
                        nc.vector.tensor_reduce(
                            u_out[:, m:m + 1], up[:, m:m + 1, :],
                            mybir.AxisListType.X, ALU.add,
                        )

                def outwrite(m, psums):
                    mc = slice(m * 128, (m + 1) * 128)
                    for n in range(NCH):
                        ot = ou.tile([128, NF], mdt.float32, tag="ot",
                                     name=f"ot{m}_{n}")
                        nc.scalar.activation(ot, psums[n], ACTF.Copy,
                                             bias=0.0, scale=2.0)
                        nc.vector.tensor_scalar_add(ot, ot, c4[:, m:m + 1])
                        nc.sync.dma_start(outT[mc, n * NF:(n + 1) * NF], ot)

                for m in range(MT):
                    psums = mains(m)
                    if l == 3:
                        outwrite(m, psums)
                        continue
                    if m < NDEF:
                        # defer: spill to SBUF (ACT), free the banks; t_sc
                        # (which needs the u AllReduce) is not ready yet
                        for n in range(NCH):
                            nc.scalar.copy(zdefs[m][:, n * NF:(n + 1) * NF],
                                           psums[n])
                    elif m == NDEF:
                        held = psums
                        # t rows: 2-col stationary digs x resident W~ slices
                        ptl = pz.tile([128, 512], mdt.float32, tag="pz",
                                      name=f"pt{l}")
                        for c in range(KT):
                            dg = dig[0] if c < KT // 2 else dig[1]
                            dc = c if c < KT // 2 else c - KT // 2
                            for s in range(NS):
                                nc.tensor.matmul(
                                    ptl[32 * s:32 * s + 2, :],
                                    dg[:, dc, :],
                                    W8[c // 2][:, c % 2, 512 * s:512 * s + 512],
                                    start=(c == 0), stop=(c == KT - 1),
                                    skip_group_check=True,
                                    tile_position=(0, 32 * s),
                                )
                        # combine+shuffle: psum rows -> sbuf -> sbuf [128, MT]
                        tsb = stat.tile([128, 512], mdt.float32, tag="tsb")
                        nc.vector.tensor_copy(out=tsb, in_=ptl)
                        t2a = stat.tile([128, MT], mdt.float32, tag=f"t2a{l}")
                        t2b = stat.tile([128, MT], mdt.float32, tag=f"t2b{l}")
                        for r, t2x in enumerate((t2a, t2b)):
                            for s in range(NS):
                                nc.gpsimd.dma_start(
                                    tscr[l][r, s, :],
                                    tsb[32 * s + r:32 * s + r + 1, :],
                                )
                            nc.gpsimd.dma_start(
                                t2x,
                                tscr[l][r:r + 1]
                                .rearrange("o g (q p) -> p (o g q)", p=128),
                            )
                        # t = (row0 + row1) / 16384
                        nc.vector.tensor_tensor(
                            out=t_sc, in0=t2a, in1=t2b, op=ALU.add,
                        )
                        nc.vector.tensor_scalar_mul(t_sc, t_sc, 1.0 / B)
                        # held psums drain first (frees banks); the SBUF
                        # spills are spread across later blocks (below) so
                        # they don't burst ahead of live psum drains
                        swrite(NDEF, held)
                        pending = list(range(NDEF))
                    else:
                        swrite(m, psums)
                        if l < 3 and pending:
                            md = pending.pop(0)
                            swrite(md, [zdefs[md][:, n * NF:(n + 1) * NF]
                                        for n in range(NCH)])
                    if halfar is not None and m == MT // 2 - 1:
                        halfar(0)
                if halfar is not None:
                    halfar(1)

            def alloc_s(pool, H, nm):
                return [
                    pool.tile([128, 2, BL], FP8, tag="s", name=f"{nm}_{i}")
                    for i in range(H // 256)
                ]

            # layer 1
            S1 = alloc_s(sB, H1, "s1")
            u1 = stat.tile([128, H1 // 128], mdt.float32, tag="u1")
            dig1 = [None, None]
            layer(0, B0, W8_1, (dig0a, dig0b), S1, u1,
                  halfar=lambda h: dig1.__setitem__(h, allreduce_u_half(1, u1, h)))

            # layer 2 (W3 prep overlaps)
            W8_3 = [prep_w_pair(wA, 2, kp, "act")
                    for kp in range(H2 // 256)]
            S2 = alloc_s(sA, H2, "s2")
            u2 = stat.tile([128, H2 // 128], mdt.float32, tag="u2")
            dig2 = [None, None]
            layer(1, S1, W8_2, dig1, S2, u2,
                  halfar=lambda h: dig2.__setitem__(h, allreduce_u_half(2, u2, h)))

            # layer 3 (W4 prep overlaps)
            W8_4 = [prep_w_pair(wB, 3, kp, "act")
                    for kp in range(H3 // 256)]
            S3 = alloc_s(sB, H3, "s3")
            layer(2, S2, W8_3, dig2, S3, None)

            # rW4 = rowsum(sign(W4)) via ones-stationary; c4 = sb4 - rW4
            ones1 = stat.tile([128, 1], mdt.float16, tag="ones1")
            nc.vector.memset(ones1, 1.0)
            ptw = pz.tile([128, 512], mdt.float32, tag="pz", name="ptw")
            KT4 = H3 // 128
            for c in range(KT4):
                nc.tensor.matmul(
                    ptw[0:1, :], ones1, W8_4[c // 2][:, c % 2, :],
                    start=(c == 0), stop=(c == KT4 - 1),
                    skip_group_check=True,
                )
            rwsb = stat.tile([128, 512], mdt.float32, tag="rwsb")
            nc.vector.tensor_copy(out=rwsb[0:1, :], in_=ptw[0:1, :])
            nc.gpsimd.dma_start(rwscr[0, :], rwsb[0:1, :])
            rw4 = stat.tile([128, C // 128], mdt.float32, tag="rw4")
            nc.gpsimd.dma_start(
                rw4, rwscr.rearrange("o (m p) -> p (o m)", p=128),
            )
            c4 = stat.tile([128, C // 128], mdt.float32, tag="c4")
            nc.vector.tensor_tensor(out=c4, in0=sb4, in1=rw4, op=ALU.subtract)

            # layer 4 (no BN): out = 2*p - rW4 + sign(b4)
            layer(3, S3, W8_4, None, None, None)

    nc.compile()
    return nc


def _get_module():
    if "nc" not in _CACHE:
        _CACHE["nc"] = _build_module()
    return _CACHE["nc"]


def _reference_fallback(x, W1, b1, g1, be1, W2, b2, g2, be2, W3, b3, g3, be3, W4, b4):
    """Exact numpy clone of the reference for non-trivial gamma/beta inputs."""
    EPS = 1e-5

    def binarize(v):
        return np.where(v >= 0, 1.0, -1.0).astype(np.float32)

    def bin_linear(a, W, b):
        return binarize(a) @ binarize(W).T + binarize(b)

    def bn(z, g, be):
        m = z.mean(axis=0)
        v = z.var(axis=0)
        return (z - m) / np.sqrt(v + EPS) * g + be

    h = np.clip(bn(bin_linear(x, W1, b1), g1, be1), -1.0, 1.0)
    h = np.clip(bn(bin_linear(h, W2, b2), g2, be2), -1.0, 1.0)
    h = np.clip(bn(bin_linear(h, W3, b3), g3, be3), -1.0, 1.0)
    return bin_linear(h, W4, b4).astype(np.float32)


def make_in_maps(inputs):
    bf16 = ml_dtypes.bfloat16
    x = inputs["x"]
    common = {
        "w1t": np.ascontiguousarray(np.asarray(inputs["W1"]).T).astype(bf16),
        "w2t": np.ascontiguousarray(np.asarray(inputs["W2"]).T).astype(bf16),
        "w3t": np.ascontiguousarray(np.asarray(inputs["W3"]).T).astype(bf16),
        "w4t": np.ascontiguousarray(np.asarray(inputs["W4"]).T).astype(bf16),
        "b4": np.asarray(inputs["b4"], dtype=np.float32).reshape(C, 1),
    }
    in_maps = []
    for c in range(NCORES):
        m = dict(common)
        m["xT"] = np.ascontiguousarray(
            np.asarray(x[c * BL:(c + 1) * BL, :]).T
        ).astype(bf16)
        in_maps.append(m)
    return in_maps


def gather_output(results):
    out = np.empty((B, C), dtype=np.float32)
    for c in range(NCORES):
        out[c * BL:(c + 1) * BL, :] = results[c]["outT"].T
    return out


def kernel(**inputs):
    # BN gamma/beta must be trivial for the sign-reduction; spec fills guarantee
    # this (g=ones, be=zeros).  Anything else falls back to exact host compute.
    for gk, bek in (("g1", "be1"), ("g2", "be2"), ("g3", "be3")):
        if not (np.all(np.asarray(inputs[gk]) == 1.0)
                and np.all(np.asarray(inputs[bek]) == 0.0)):
            return _reference_fallback(**{
                k: np.asarray(v, dtype=np.float32) for k, v in inputs.items()
            })

    from concourse.bass_utils import run_bass_kernel_spmd

    nc = _get_module()
    in_maps = make_in_maps(inputs)
    res = run_bass_kernel_spmd(nc, in_maps, list(range(NCORES)))
    return gather_output(res.results)


if __name__ == "__main__":
    nc = _get_module()
    print("module built OK")



# revision 2
# speedup vs baseline: 1.0250x; 1.0250x over previous
"""Trainium2 Bass kernel for nn_NeuralNetworkSimplified (binarized 4-layer MLP + BN).

Math
----
reference computes, per hidden layer l (gamma=1, beta=0, biases b_l arbitrary):
    z = sign(a) @ sign(W).T + sign(b)
    h = clip(batchnorm_train(z), -1, 1)
and the next layer only consumes sign(h).  Since batchnorm's rsqrt(var+eps) > 0
and gamma=1/beta=0, sign(h) = sign(z - mean_batch(z)); the sign(b) bias shifts
z and its mean equally, so it cancels.  The network reduces to exact integer
arithmetic.  On device we use an asymmetric encoding:
    W~ = sign(W) in {-1, +1}   (fp8, exact)
    B  = 1{a >= 0} in {0, 1}   (fp8, exact)
    p[m,n] = sum_k W~[k,m] * B[k,n]        (psum, exact ints)
    true Z = 2p - rowsum(W~)[m]; the rowsum term is constant per feature m, so
    it cancels in the batch-mean comparison:
        sign(Z - colmean(Z)) = 1{p >= pmean},  pmean = (W~ @ u) / 16384,
    where u[k] = global colsum of B (AllReduce of per-core counts).
The threshold pmean is computed on the PE with a tiny 2-column stationary
[u - 8192 (fp16 exact, |.| <= 2048), 8192] against the already-resident W~
tiles (moving operand), using 32-aligned column groups of one PSUM bank.
The final layer outputs 2*p4 - rowsum(W4~) + sign(b4) (exact ints, fp16-safe).

Host prep is layout-only: inputs ship as the TOP BYTE of each fp32 value
(a pure bit-slice), reinterpreted as fp8e5 -- the sign is preserved exactly
and the on-device sign()/1{>=0} ops see a value of identical sign, so all
FLOPs still run on device while input DMA bytes are halved.

Scheduling (v2):
 - per hidden layer, blocks m=0..4 spill psum->SBUF(fp16, exact) via ACT so
   the PE never waits on the batch-mean AllReduce; threshold matmuls run as
   two accumulation groups (first K-half after m=3's mains, second after
   m=4's) giving each 4KB AllReduce a ~35us grace window against its
   ~20-33us latency.
 - u AllReduce halves fire as soon as their feature columns complete
   (after the m=9 drain), not at layer end.
 - layer 4 writes 2p+c4 in one fused DVE op to fp16 output tiles.
"""

import numpy as np
import ml_dtypes

B, D, H1, H2, H3, C = 16384, 3072, 2048, 2048, 1024, 512
NCORES = 8
BL = B // NCORES          # 2048 rows per core
NF = 512                  # batch free-dim chunk (one psum bank)
NCH = BL // NF            # 4 chunks
LAYERS = [(D, H1), (H1, H2), (H2, H3), (H3, C)]
UDIMS = [D, H1, H2]       # length of u vector feeding each hidden layer's mean

_CACHE = {}


def _build_module():
    import concourse.bass as bass
    import concourse.mybir as mybir
    import concourse.tile as tile
    from concourse import bacc

    mdt = mybir.dt
    FP8 = mdt.float8e4
    RAW = mdt.float8e5          # byte-truncated fp32: sign-exact
    ALU = mybir.AluOpType
    ACTF = mybir.ActivationFunctionType
    DR = mybir.MatmulPerfMode.DoubleRow

    nc = bacc.Bacc(
        "TRN2",
        target_bir_lowering=False,
        debug=False,
        num_devices=NCORES,
    )

    xT = nc.dram_tensor("xT", [D, BL], RAW, kind="ExternalInput").ap()
    wT = [
        nc.dram_tensor(f"w{i + 1}t", [K, H], RAW, kind="ExternalInput").ap()
        for i, (K, H) in enumerate(LAYERS)
    ]
    b4 = nc.dram_tensor("b4", [C, 1], mdt.float32, kind="ExternalInput").ap()
    outT = nc.dram_tensor("outT", [C, BL], mdt.float16, kind="ExternalOutput").ap()

    cc_in = [
        [nc.dram_tensor(f"cc_in{i}_{h}", [128, U // 256], mdt.float32).ap()
         for h in range(2)]
        for i, U in enumerate(UDIMS)
    ]
    cc_out = [
        [nc.dram_tensor(f"cc_out{i}_{h}", [128, U // 256], mdt.float32,
                        addr_space="Shared").ap()
         for h in range(2)]
        for i, U in enumerate(UDIMS)
    ]
    # scratch for cross-partition shuffle of the threshold rows (t) per layer
    tscr = [
        nc.dram_tensor(f"tscr{i}", [2, H // 512, 512], mdt.float32).ap()
        for i, H in enumerate([H1, H2, H3])
    ]
    rwscr = nc.dram_tensor("rwscr", [1, C], mdt.float32).ap()
    ccw_in = nc.dram_tensor("ccw_in", [128, 1], mdt.float32).ap()
    ccw_out = nc.dram_tensor("ccw_out", [128, 1], mdt.float32, addr_space="Shared").ap()

    with tile.TileContext(nc, num_cores=NCORES) as tc:
        with (
            tc.tile_pool(name="raw", bufs=2) as raw,       # staging raw-byte tiles
            tc.tile_pool(name="sA", bufs=12) as sA,        # B0, S2 pair tiles
            tc.tile_pool(name="sB", bufs=8) as sB,         # S1, S3 pair tiles
            tc.tile_pool(name="wA", bufs=12) as wA,        # W1~, W3~ pair tiles
            tc.tile_pool(name="wB", bufs=8) as wB,         # W2~, W4~ pair tiles
            tc.tile_pool(name="stat", bufs=1) as stat,     # u/t/bias vectors
            tc.tile_pool(name="zd", bufs=5) as zd,         # deferred psum spills
            tc.tile_pool(name="ou", bufs=6) as ou,         # output staging (fp16)
            tc.tile_pool(name="pz", bufs=8, space="PSUM") as pz,
        ):
            # ---- PE warm-up while the prologue runs on DMA/DVE ----
            warm = stat.tile([128, 128], FP8, tag="warm")
            nc.vector.memset(warm, 1.0)
            wps = pz.tile([128, 128], mdt.float32, tag="pz", name="warmps")
            for i in range(36):
                nc.tensor.matmul(wps, warm, warm, start=True, stop=True)

            # warm the collective stream early: the first CC op pays ~60-90us
            # of stream setup; do it on garbage concurrently with the prologue
            ccwt = stat.tile([128, 1], mdt.float32, tag="ccwt")
            nc.vector.memset(ccwt, 0.0)
            nc.gpsimd.dma_start(ccw_in, ccwt)
            nc.gpsimd.collective_compute(
                "AllReduce", ALU.add,
                replica_groups=[list(range(NCORES))],
                ins=[ccw_in], outs=[ccw_out],
            )

            # tiny positive bias for ACT Sign: breaks w==0 ties toward +1
            biap = stat.tile([128, 1], mdt.float32, tag="biap")
            nc.vector.memset(biap, 1e-30)

            # ---- sign(b4) as +-1 per-partition vector, [128, C//128] ----
            b4_sb = stat.tile([128, C // 128], mdt.float32, tag="rawb")
            nc.sync.dma_start(b4_sb, b4.rearrange("(o p) q -> p (o q)", p=128))
            sb4 = stat.tile([128, C // 128], mdt.float32, tag="sb4")
            nc.gpsimd.tensor_scalar(
                out=sb4, in0=b4_sb, scalar1=0.0, scalar2=2.0,
                op0=ALU.is_ge, op1=ALU.mult,
            )
            nc.gpsimd.tensor_scalar_add(sb4, sb4, -1.0)

            # ---- binarize helpers ----
            def prep_w_pair(pool, idx, kp):
                """raw-byte dram pair -> fp8 +-1 pair tile (sign, exact)."""
                K, H = LAYERS[idx]
                w8 = pool.tile([128, 2, H], FP8, tag="w", name=f"w{idx}_{kp}")
                for h in range(2):
                    rt = raw.tile([128, H], RAW, tag="raww",
                                  name=f"rw{idx}_{kp}_{h}")
                    nc.sync.dma_start(
                        rt,
                        wT[idx][(2 * kp + h) * 128:(2 * kp + h + 1) * 128, :],
                    )
                    nc.scalar.activation(w8[:, h, :], rt,
                                         ACTF.Sign, bias=biap, scale=1.0)
                return w8

            # ---- u AllReduce, half-granular ----
            def allreduce_u_half(idx, u_sb, half):
                HC = UDIMS[idx] // 256
                sl = slice(half * HC, (half + 1) * HC)
                nc.gpsimd.dma_start(cc_in[idx][half], u_sb[:, sl])
                nc.gpsimd.collective_compute(
                    "AllReduce",
                    ALU.add,
                    replica_groups=[list(range(NCORES))],
                    ins=[cc_in[idx][half]],
                    outs=[cc_out[idx][half]],
                )
                ug = stat.tile([128, HC], mdt.float32, tag=f"ug{idx}_{half}")
                nc.gpsimd.dma_start(ug, cc_out[idx][half])
                # dig[:, c, 0] = u - 8192 (exact fp16, |.| <= 2048), dig[:, c, 1] = 8192
                dig = stat.tile([128, HC, 2], mdt.float16, tag=f"dig{idx}_{half}")
                nc.vector.memset(dig[:, :, 1:2], 8192.0)
                nc.vector.tensor_scalar_add(dig[:, :, 0:1], ug.unsqueeze(2), -8192.0)
                return dig

            # ---- x prep: B0 = 1{x>=0} (fp8 {0,1}) + u0 counts fused ----
            B0 = []
            u0 = stat.tile([128, D // 128], mdt.float32, tag="u0")
            # W1 pair 0 first so L1-m0's first matmuls start ASAP
            W8_1 = [prep_w_pair(wA, 0, 0)]
            for kp in range(D // 256):
                s8 = sA.tile([128, 2, BL], FP8, tag="s", name=f"s0_{kp}")
                for h in range(2):
                    rt = raw.tile([128, BL], RAW, tag="rawx",
                                  name=f"rx{kp}_{h}")
                    nc.sync.dma_start(
                        rt,
                        xT[(2 * kp + h) * 128:(2 * kp + h + 1) * 128, :],
                    )
                    nc.vector.tensor_scalar(
                        out=s8[:, h, :], in0=rt, scalar1=0.0,
                        scalar2=0.0, op0=ALU.is_ge, op1=ALU.add,
                        accum_out=u0[:, 2 * kp + h:2 * kp + h + 1],
                    )
                B0.append(s8)
                # x leads W1 2:1 so u0 (and its AllReduce) finish early;
                # W1's tail overlaps L1 m0's streamed matmuls
                if kp % 2 == 1 and kp // 2 + 1 < D // 512:
                    W8_1.append(prep_w_pair(wA, 0, kp // 2 + 1))
                if kp == D // 512 - 1:
                    dig0a = allreduce_u_half(0, u0, 0)
            dig0b = allreduce_u_half(0, u0, 1)
            for kp in range(D // 512, D // 256):
                W8_1.append(prep_w_pair(wA, 0, kp))

            W8_2 = [prep_w_pair(wB, 1, kp) for kp in range(H1 // 256)]

            # ---- one layer ----
            # hidden layers: blocks m<ND2 spill psum->SBUF fp16 (exact: p is a
            # small count) so the PE never stalls on the threshold chain; the
            # t matmuls accumulate in two groups (K-half each) to tolerate the
            # AllReduce latency of the second dig half.
            ND2, TA = 5, 3

            def layer(l, A8, W8, dig, S_out, u_out, halfar=None):
                K, H = LAYERS[l]
                KT, MT = K // 128, H // 128
                NS = H // 512          # 512-wide feature slices for the t rows
                t_sc = stat.tile([128, MT], mdt.float32, tag=f"tsc{l}",
                                 name=f"tsc{l}") if l < 3 else None
                up = stat.tile([128, MT, NCH], mdt.float32, tag=f"up{l}",
                               name=f"up{l}") if l < 3 else None
                zdefs = [zd.tile([128, BL], mdt.float16, tag="zd",
                                 name=f"zd{l}_{m}") for m in range(ND2)] \
                    if l < 3 else None
                ptl = [None]
                pending = []           # spilled blocks awaiting their S-write

                def mains(m):
                    mc = slice(m * 128, (m + 1) * 128)
                    psums = [
                        pz.tile([128, NF], mdt.float32, tag="pz",
                                name=f"pz{l}_{m}_{n}")
                        for n in range(NCH)
                    ]
                    for kp in range(KT // 2):
                        st, sp = kp == 0, kp == KT // 2 - 1
                        wsl = W8[kp][:, :, mc]
                        for n in range(NCH):
                            nc.tensor.matmul(
                                psums[n], wsl, A8[kp][:, :, n * NF:(n + 1) * NF],
                                start=st, stop=sp, perf_mode=DR,
                            )
                    return psums

                def tmm_group(grp):
                    # threshold rows: 2-col stationary digs x resident W~
                    # slices; grp 0 = first K-half (dig[0]), 1 = second.
                    if grp == 0:
                        ptl[0] = pz.tile([128, 512], mdt.float32, tag="pz",
                                         name=f"pt{l}")
                    c0, c1 = (0, KT // 2) if grp == 0 else (KT // 2, KT)
                    dg = dig[grp]
                    for c in range(c0, c1):
                        dc = c - c0
                        for s in range(NS):
                            nc.tensor.matmul(
                                ptl[0][32 * s:32 * s + 2, :],
                                dg[:, dc, :],
                                W8[c // 2][:, c % 2, 512 * s:512 * s + 512],
                                start=(c == 0), stop=(c == KT - 1),
                                skip_group_check=True,
                                tile_position=(0, 32 * s),
                            )

                def t_combine():
                    # combine+shuffle: psum rows -> sbuf -> sbuf [128, MT]
                    tsb = stat.tile([128, 512], mdt.float32, tag="tsb")
                    nc.vector.tensor_copy(out=tsb, in_=ptl[0])
                    t2a = stat.tile([128, MT], mdt.float32, tag=f"t2a{l}")
                    t2b = stat.tile([128, MT], mdt.float32, tag=f"t2b{l}")
                    for r, t2x in enumerate((t2a, t2b)):
                        for s in range(NS):
                            nc.gpsimd.dma_start(
                                tscr[l][r, s, :],
                                tsb[32 * s + r:32 * s + r + 1, :],
                            )
                        nc.gpsimd.dma_start(
                            t2x,
                            tscr[l][r:r + 1]
                            .rearrange("o g (q p) -> p (o g q)", p=128),
                        )
                    # t = (row0 + row1) / 16384
                    nc.vector.tensor_tensor(
                        out=t_sc, in0=t2a, in1=t2b, op=ALU.add,
                    )
                    nc.vector.tensor_scalar_mul(t_sc, t_sc, 1.0 / B)

                def swrite(m, srcs, eng=None):
                    # S_out = 1{p >= t}; u partials fused via accum_out
                    e = eng or nc.vector
                    for n in range(NCH):
                        if up is not None and u_out is not None:
                            e.tensor_scalar(
                                out=S_out[m // 2][:, m % 2, n * NF:(n + 1) * NF],
                                in0=srcs[n], scalar1=t_sc[:, m:m + 1],
                                scalar2=0.0, op0=ALU.is_ge, op1=ALU.add,
                                accum_out=up[:, m, n:n + 1],
                            )
                        else:
                            e.tensor_scalar(
                                out=S_out[m // 2][:, m % 2, n * NF:(n + 1) * NF],
                                in0=srcs[n], scalar1=t_sc[:, m:m + 1],
                                scalar2=0.0, op0=ALU.is_ge, op1=ALU.add,
                            )
                    if u_out is not None:
                        nc.vector.tensor_reduce(
                            u_out[:, m:m + 1], up[:, m:m + 1, :],
                            mybir.AxisListType.X, ALU.add,
                        )

                def drain_one(i=0):
                    md = pending.pop(0)
                    # L3 has no u accumulation -> gpsimd can share the drains
                    eng = nc.gpsimd if (u_out is None and i % 2 == 1) else None
                    swrite(md, [zdefs[md][:, n * NF:(n + 1) * NF]
                                for n in range(NCH)], eng=eng)

                for m in range(MT):
                    psums = mains(m)
                    if l == 3:
                        # out = 2*p + (sign(b4) - rowsum(W4~)) fused on DVE
                        mc = slice(m * 128, (m + 1) * 128)
                        for n in range(NCH):
                            ot = ou.tile([128, NF], mdt.float16, tag="ot",
                                         name=f"ot{m}_{n}")
                            nc.vector.tensor_scalar(
                                out=ot, in0=psums[n], scalar1=2.0,
                                scalar2=c4[:, m:m + 1],
                                op0=ALU.mult, op1=ALU.add,
                            )
                            nc.sync.dma_start(outT[mc, n * NF:(n + 1) * NF], ot)
                        continue
                    if m < ND2:
                        # spill to SBUF (ACT, fp16 exact) and free the banks;
                        # t_sc (needs the u AllReduce) is not ready yet
                        for n in range(NCH):
                            nc.scalar.copy(zdefs[m][:, n * NF:(n + 1) * NF],
                                           psums[n])
                        pending.append(m)
                    if m == TA:
                        tmm_group(0)
                    if m == ND2 - 1:
                        tmm_group(1)
                        t_combine()
                    if m >= ND2:
                        swrite(m, psums)
                        ndr = 1 if u_out is not None else 2
                        for i in range(ndr):
                            if pending:
                                drain_one(i)
                    if halfar is not None and m == 9:
                        halfar(0)
                while pending:
                    drain_one()
                if halfar is not None:
                    halfar(1)

            def alloc_s(pool, H, nm):
                return [
                    pool.tile([128, 2, BL], FP8, tag="s", name=f"{nm}_{i}")
                    for i in range(H // 256)
                ]

            # layer 1
            S1 = alloc_s(sB, H1, "s1")
            u1 = stat.tile([128, H1 // 128], mdt.float32, tag="u1")
            dig1 = [None, None]
            layer(0, B0, W8_1, (dig0a, dig0b), S1, u1,
                  halfar=lambda h: dig1.__setitem__(h, allreduce_u_half(1, u1, h)))

            # layer 2 (W3 prep overlaps)
            W8_3 = [prep_w_pair(wA, 2, kp) for kp in range(H2 // 256)]
            S2 = alloc_s(sA, H2, "s2")
            u2 = stat.tile([128, H2 // 128], mdt.float32, tag="u2")
            dig2 = [None, None]
            layer(1, S1, W8_2, dig1, S2, u2,
                  halfar=lambda h: dig2.__setitem__(h, allreduce_u_half(2, u2, h)))

            # W4 prep + rW4 = rowsum(sign(W4)) early so L4 never waits;
            # c4 = sb4 - rW4
            W8_4 = [prep_w_pair(wB, 3, kp) for kp in range(H3 // 256)]
            ones1 = stat.tile([128, 1], mdt.float16, tag="ones1")
            nc.vector.memset(ones1, 1.0)
            ptw = pz.tile([128, 512], mdt.float32, tag="pz", name="ptw")
            KT4 = H3 // 128
            for c in range(KT4):
                nc.tensor.matmul(
                    ptw[0:1, :], ones1, W8_4[c // 2][:, c % 2, :],
                    start=(c == 0), stop=(c == KT4 - 1),
                    skip_group_check=True,
                )
            rwsb = stat.tile([128, 512], mdt.float32, tag="rwsb")
            nc.vector.tensor_copy(out=rwsb[0:1, :], in_=ptw[0:1, :])
            nc.gpsimd.dma_start(rwscr[0, :], rwsb[0:1, :])
            rw4 = stat.tile([128, C // 128], mdt.float32, tag="rw4")
            nc.gpsimd.dma_start(
                rw4, rwscr.rearrange("o (m p) -> p (o m)", p=128),
            )
            c4 = stat.tile([128, C // 128], mdt.float32, tag="c4")
            nc.vector.tensor_tensor(out=c4, in0=sb4, in1=rw4, op=ALU.subtract)

            # layer 3
            S3 = alloc_s(sB, H3, "s3")
            layer(2, S2, W8_3, dig2, S3, None)

            # layer 4 (no BN): out = 2*p - rW4 + sign(b4)
            layer(3, S3, W8_4, None, None, None)

    nc.compile()
    return nc


def _get_module():
    if "nc" not in _CACHE:
        _CACHE["nc"] = _build_module()
    return _CACHE["nc"]


def _reference_fallback(x, W1, b1, g1, be1, W2, b2, g2, be2, W3, b3, g3, be3, W4, b4):
    """Exact numpy clone of the reference for non-trivial gamma/beta inputs."""
    EPS = 1e-5

    def binarize(v):
        return np.where(v >= 0, 1.0, -1.0).astype(np.float32)

    def bin_linear(a, W, b):
        return binarize(a) @ binarize(W).T + binarize(b)

    def bn(z, g, be):
        m = z.mean(axis=0)
        v = z.var(axis=0)
        return (z - m) / np.sqrt(v + EPS) * g + be

    h = np.clip(bn(bin_linear(x, W1, b1), g1, be1), -1.0, 1.0)
    h = np.clip(bn(bin_linear(h, W2, b2), g2, be2), -1.0, 1.0)
    h = np.clip(bn(bin_linear(h, W3, b3), g3, be3), -1.0, 1.0)
    return bin_linear(h, W4, b4).astype(np.float32)


def _topbyte(a):
    """fp32 array -> top byte (sign+exponent msbs) as fp8e5: sign-exact."""
    a = np.ascontiguousarray(np.asarray(a, dtype=np.float32))
    return (a.view(np.uint32) >> 24).astype(np.uint8).view(ml_dtypes.float8_e5m2)


def make_in_maps(inputs):
    x8 = _topbyte(inputs["x"])
    common = {
        "w1t": np.ascontiguousarray(_topbyte(inputs["W1"]).T),
        "w2t": np.ascontiguousarray(_topbyte(inputs["W2"]).T),
        "w3t": np.ascontiguousarray(_topbyte(inputs["W3"]).T),
        "w4t": np.ascontiguousarray(_topbyte(inputs["W4"]).T),
        "b4": np.asarray(inputs["b4"], dtype=np.float32).reshape(C, 1),
    }
    in_maps = []
    for c in range(NCORES):
        m = dict(common)
        m["xT"] = np.ascontiguousarray(x8[c * BL:(c + 1) * BL, :].T)
        in_maps.append(m)
    return in_maps


def gather_output(results):
    out = np.empty((B, C), dtype=np.float32)
    for c in range(NCORES):
        out[c * BL:(c + 1) * BL, :] = results[c]["outT"].T.astype(np.float32)
    return out


def kernel(**inputs):
    # BN gamma/beta must be trivial for the sign-reduction; spec fills guarantee
    # this (g=ones, be=zeros).  Anything else falls back to exact host compute.
    for gk, bek in (("g1", "be1"), ("g2", "be2"), ("g3", "be3")):
        if not (np.all(np.asarray(inputs[gk]) == 1.0)
                and np.all(np.asarray(inputs[bek]) == 0.0)):
            return _reference_fallback(**{
                k: np.asarray(v, dtype=np.float32) for k, v in inputs.items()
            })

    from concourse.bass_utils import run_bass_kernel_spmd

    nc = _get_module()
    in_maps = make_in_maps(inputs)
    res = run_bass_kernel_spmd(nc, in_maps, list(range(NCORES)))
    return gather_output(res.results)


if __name__ == "__main__":
    nc = _get_module()
    print("module built OK")


# revision 13
# speedup vs baseline: 1.0422x; 1.0168x over previous
"""Trainium2 Bass kernel for nn_NeuralNetworkSimplified (binarized 4-layer MLP + BN).

Math
----
reference computes, per hidden layer l (gamma=1, beta=0, biases b_l arbitrary):
    z = sign(a) @ sign(W).T + sign(b)
    h = clip(batchnorm_train(z), -1, 1)
and the next layer only consumes sign(h).  Since batchnorm's rsqrt(var+eps) > 0
and gamma=1/beta=0, sign(h) = sign(z - mean_batch(z)); the sign(b) bias shifts
z and its mean equally, so it cancels.  The network reduces to exact integer
arithmetic.  On device we use an asymmetric encoding:
    W~ = sign(W) in {-1, +1}   (fp8, exact)
    B  = 1{a >= 0} in {0, 1}   (fp8, exact)
    p[m,n] = sum_k W~[k,m] * B[k,n]        (psum, exact ints)
    true Z = 2p - rowsum(W~)[m]; the rowsum term is constant per feature m, so
    it cancels in the batch-mean comparison:
        sign(Z - colmean(Z)) = 1{p >= pmean},  pmean = (W~ @ u) / 16384,
    where u[k] = global colsum of B (AllReduce of per-core counts).
The threshold pmean is computed on the PE with a tiny 2-column stationary
[u - 8192 (fp16 exact, |.| <= 2048), 8192] against the already-resident W~
tiles (moving operand), in 32-aligned column groups of one PSUM bank.
The final layer outputs 2*p4 - rowsum(W4~) + sign(b4).

Host prep is layout-only: inputs ship as the TOP BYTE of each fp32 value
(a pure bit-slice), reinterpreted as fp8e5 -- the sign is preserved exactly,
halving input DMA; all FLOPs including every sign() run on device.

Scheduling (v4):
 - per hidden layer, blocks m=0..4 spill psum->SBUF(fp16, exact small counts)
   via ACT so the PE never waits on the batch-mean AllReduce; the threshold
   matmuls run as two accumulation groups (first K-half after m=3's mains,
   second after m=4's), giving each 4KB AllReduce (~20-33us latency) a ~35us
   grace window.
 - u AllReduce halves fire as soon as their feature columns complete (after
   the m=9 drain); the dig (threshold stationary) builds are EMITTED at the
   consuming layer so they never sit at an engine queue head blocking
   unrelated work on the AllReduce.
 - S-writes and drains always use the DVE accum_out path (the plain
   tensor_scalar path measures ~4x slower); drains are single 2048-wide ops
   producing their u column directly.
 - the t-row shuffle runs its two row-chains on sync and gpsimd in parallel.
"""

import numpy as np
import ml_dtypes

B, D, H1, H2, H3, C = 16384, 3072, 2048, 2048, 1024, 512
NCORES = 8
BL = B // NCORES          # 2048 rows per core
NF = 512                  # batch free-dim chunk (one psum bank)
NCH = BL // NF            # 4 chunks
LAYERS = [(D, H1), (H1, H2), (H2, H3), (H3, C)]
UDIMS = [D, H1, H2]       # length of u vector feeding each hidden layer's mean

_CACHE = {}


def _build_module():
    import concourse.bass as bass
    import concourse.mybir as mybir
    import concourse.tile as tile
    from concourse import bacc

    mdt = mybir.dt
    FP8 = mdt.float8e4
    RAW = mdt.float8e5          # byte-truncated fp32: sign-exact
    ALU = mybir.AluOpType
    ACTF = mybir.ActivationFunctionType
    DR = mybir.MatmulPerfMode.DoubleRow

    nc = bacc.Bacc(
        "TRN2",
        target_bir_lowering=False,
        debug=False,
        num_devices=NCORES,
    )

    xT = nc.dram_tensor("xT", [D, BL], RAW, kind="ExternalInput").ap()
    wT = [
        nc.dram_tensor(f"w{i + 1}t", [K, H], RAW, kind="ExternalInput").ap()
        for i, (K, H) in enumerate(LAYERS)
    ]
    b4 = nc.dram_tensor("b4", [C, 1], mdt.float32, kind="ExternalInput").ap()
    outT = nc.dram_tensor("outT", [C, BL], mdt.float32, kind="ExternalOutput").ap()

    cc_in = [
        [nc.dram_tensor(f"cc_in{i}_{h}", [128, U // 256], mdt.float32).ap()
         for h in range(2)]
        for i, U in enumerate(UDIMS)
    ]
    cc_out = [
        [nc.dram_tensor(f"cc_out{i}_{h}", [128, U // 256], mdt.float32,
                        addr_space="Shared").ap()
         for h in range(2)]
        for i, U in enumerate(UDIMS)
    ]
    # scratch for cross-partition shuffle of the threshold rows (t) per layer
    tscr = [
        nc.dram_tensor(f"tscr{i}", [2, H // 512, 512], mdt.float32).ap()
        for i, H in enumerate([H1, H2, H3])
    ]
    rwscr = nc.dram_tensor("rwscr", [1, C], mdt.float32).ap()
    ccw_in = nc.dram_tensor("ccw_in", [128, 1], mdt.float32).ap()
    ccw_out = nc.dram_tensor("ccw_out", [128, 1], mdt.float32, addr_space="Shared").ap()

    with tile.TileContext(nc, num_cores=NCORES) as tc:
        with (
            tc.tile_pool(name="raw", bufs=2) as raw,       # staging raw-byte tiles
            tc.tile_pool(name="sA", bufs=12) as sA,        # B0, S2 pair tiles
            tc.tile_pool(name="sB", bufs=8) as sB,         # S1, S3 pair tiles
            tc.tile_pool(name="wA", bufs=12) as wA,        # W1~, W3~ pair tiles
            tc.tile_pool(name="wB", bufs=8) as wB,         # W2~, W4~ pair tiles
            tc.tile_pool(name="stat", bufs=1) as stat,     # u/t/bias vectors
            tc.tile_pool(name="zd", bufs=5) as zd,         # deferred psum spills
            tc.tile_pool(name="ou", bufs=4) as ou,         # output staging
            tc.tile_pool(name="pz", bufs=8, space="PSUM") as pz,
        ):
            # ---- PE warm-up while the prologue runs on DMA/DVE ----
            warm = stat.tile([128, 128], FP8, tag="warm")
            nc.vector.memset(warm, 1.0)
            wps = pz.tile([128, 128], mdt.float32, tag="pz", name="warmps")
            for i in range(36):
                nc.tensor.matmul(wps, warm, warm, start=True, stop=True)

            # warm the collective stream early
            ccwt = stat.tile([128, 1], mdt.float32, tag="ccwt")
            nc.vector.memset(ccwt, 0.0)
            nc.gpsimd.dma_start(ccw_in, ccwt)
            nc.gpsimd.collective_compute(
                "AllReduce", ALU.add,
                replica_groups=[list(range(NCORES))],
                ins=[ccw_in], outs=[ccw_out],
            )

            # tiny positive bias for ACT Sign: breaks w==0 ties toward +1
            biap = stat.tile([128, 1], mdt.float32, tag="biap")
            nc.vector.memset(biap, 1e-30)

            # ---- sign(b4) as +-1 per-partition vector, [128, C//128] ----
            b4_sb = stat.tile([128, C // 128], mdt.float32, tag="rawb")
            nc.sync.dma_start(b4_sb, b4.rearrange("(o p) q -> p (o q)", p=128))
            sb4 = stat.tile([128, C // 128], mdt.float32, tag="sb4")
            nc.gpsimd.tensor_scalar(
                out=sb4, in0=b4_sb, scalar1=0.0, scalar2=2.0,
                op0=ALU.is_ge, op1=ALU.mult,
            )
            nc.gpsimd.tensor_scalar_add(sb4, sb4, -1.0)

            # ---- binarize helpers ----
            def prep_w_pair(pool, idx, kp):
                """raw-byte dram pair -> fp8 +-1 pair tile (sign, exact)."""
                K, H = LAYERS[idx]
                w8 = pool.tile([128, 2, H], FP8, tag="w", name=f"w{idx}_{kp}")
                for h in range(2):
                    rt = raw.tile([128, H], RAW, tag="raww",
                                  name=f"rw{idx}_{kp}_{h}")
                    nc.sync.dma_start(
                        rt,
                        wT[idx][(2 * kp + h) * 128:(2 * kp + h + 1) * 128, :],
                    )
                    nc.scalar.activation(w8[:, h, :], rt,
                                         ACTF.Sign, bias=biap, scale=1.0)
                return w8

            # ---- u AllReduce fire (dig build deferred to the consumer) ----
            def ar_fire(idx, u_sb, half):
                HC = UDIMS[idx] // 256
                sl = slice(half * HC, (half + 1) * HC)
                nc.gpsimd.dma_start(cc_in[idx][half], u_sb[:, sl])
                nc.gpsimd.collective_compute(
                    "AllReduce",
                    ALU.add,
                    replica_groups=[list(range(NCORES))],
                    ins=[cc_in[idx][half]],
                    outs=[cc_out[idx][half]],
                )

            def dig_build(idx, half):
                """AR result -> dig stationary [128, HC, 2] (fp16 exact)."""
                HC = UDIMS[idx] // 256
                ug = stat.tile([128, HC], mdt.float32, tag=f"ug{idx}_{half}")
                nc.gpsimd.dma_start(ug, cc_out[idx][half])
                dig = stat.tile([128, HC, 2], mdt.float16, tag=f"dig{idx}_{half}")
                nc.vector.memset(dig[:, :, 1:2], 8192.0)
                nc.vector.tensor_scalar_add(dig[:, :, 0:1], ug.unsqueeze(2),
                                            -8192.0)
                return dig

            # ---- x prep: B0 = 1{x>=0} (fp8 {0,1}) + u0 counts fused ----
            B0 = []
            u0 = stat.tile([128, D // 128], mdt.float32, tag="u0")
            W8_1 = [prep_w_pair(wA, 0, 0)]
            for kp in range(D // 256):
                s8 = sA.tile([128, 2, BL], FP8, tag="s", name=f"s0_{kp}")
                for h in range(2):
                    rt = raw.tile([128, BL], RAW, tag="rawx",
                                  name=f"rx{kp}_{h}")
                    nc.sync.dma_start(
                        rt,
                        xT[(2 * kp + h) * 128:(2 * kp + h + 1) * 128, :],
                    )
                    nc.vector.tensor_scalar(
                        out=s8[:, h, :], in0=rt, scalar1=0.0,
                        scalar2=0.0, op0=ALU.is_ge, op1=ALU.add,
                        accum_out=u0[:, 2 * kp + h:2 * kp + h + 1],
                    )
                B0.append(s8)
                # x leads W1 2:1 so u0 (and its AllReduce) finish early
                if kp % 2 == 1 and kp // 2 + 1 < D // 512:
                    W8_1.append(prep_w_pair(wA, 0, kp // 2 + 1))
                if kp == D // 512 - 1:
                    ar_fire(0, u0, 0)
            ar_fire(0, u0, 1)
            for kp in range(D // 512, D // 256):
                W8_1.append(prep_w_pair(wA, 0, kp))

            W8_2 = [prep_w_pair(wB, 1, kp) for kp in range(H1 // 256)]

            # ---- one layer ----
            ND2, TA = 5, 3

            def layer(l, A8, W8, uidx, S_out, u_out, halfar=None):
                K, H = LAYERS[l]
                KT, MT = K // 128, H // 128
                NS = H // 512
                t_sc = stat.tile([128, MT], mdt.float32, tag=f"tsc{l}",
                                 name=f"tsc{l}") if l < 3 else None
                up = stat.tile([128, MT, NCH], mdt.float32, tag=f"up{l}",
                               name=f"up{l}") if l < 3 else None
                zdefs = [zd.tile([128, BL], mdt.float16, tag="zd",
                                 name=f"zd{l}_{m}") for m in range(ND2)] \
                    if l < 3 else None
                ptl = [None]
                pending = []

                def mains(m):
                    mc = slice(m * 128, (m + 1) * 128)
                    psums = [
                        pz.tile([128, NF], mdt.float32, tag="pz",
                                name=f"pz{l}_{m}_{n}")
                        for n in range(NCH)
                    ]
                    for kp in range(KT // 2):
                        st, sp = kp == 0, kp == KT // 2 - 1
                        wsl = W8[kp][:, :, mc]
                        for n in range(NCH):
                            nc.tensor.matmul(
                                psums[n], wsl, A8[kp][:, :, n * NF:(n + 1) * NF],
                                start=st, stop=sp, perf_mode=DR,
                            )
                    return psums

                def tmm_group(grp):
                    if grp == 0:
                        ptl[0] = pz.tile([128, 512], mdt.float32, tag="pz",
                                         name=f"pt{l}")
                    dg = dig_build(uidx, grp)
                    c0, c1 = (0, KT // 2) if grp == 0 else (KT // 2, KT)
                    for c in range(c0, c1):
                        dc = c - c0
                        for s in range(NS):
                            nc.tensor.matmul(
                                ptl[0][32 * s:32 * s + 2, :],
                                dg[:, dc, :],
                                W8[c // 2][:, c % 2, 512 * s:512 * s + 512],
                                start=(c == 0), stop=(c == KT - 1),
                                skip_group_check=True,
                                tile_position=(0, 32 * s),
                            )

                def t_combine():
                    # psum rows -> sbuf -> [128, MT]; r=0 on sync, r=1 on
                    # gpsimd so the two shuffle chains run in parallel
                    tsb = stat.tile([128, 512], mdt.float32, tag="tsb")
                    nc.vector.tensor_copy(out=tsb, in_=ptl[0])
                    t2a = stat.tile([128, MT], mdt.float32, tag=f"t2a{l}")
                    t2b = stat.tile([128, MT], mdt.float32, tag=f"t2b{l}")
                    for r, t2x in enumerate((t2a, t2b)):
                        eng = nc.sync if r == 0 else nc.gpsimd
                        for s in range(NS):
                            eng.dma_start(
                                tscr[l][r, s, :],
                                tsb[32 * s + r:32 * s + r + 1, :],
                            )
                        eng.dma_start(
                            t2x,
                            tscr[l][r:r + 1]
                            .rearrange("o g (q p) -> p (o g q)", p=128),
                        )
                    nc.vector.tensor_tensor(
                        out=t_sc, in0=t2a, in1=t2b, op=ALU.add,
                    )
                    nc.vector.tensor_scalar_mul(t_sc, t_sc, 1.0 / B)

                def swrite_live(m, psums):
                    # S_out = 1{p >= t}; per-chunk u partials via accum_out
                    # (always accum: the plain path is ~4x slower)
                    for n in range(NCH):
                        nc.vector.tensor_scalar(
                            out=S_out[m // 2][:, m % 2, n * NF:(n + 1) * NF],
                            in0=psums[n], scalar1=t_sc[:, m:m + 1],
                            scalar2=0.0, op0=ALU.is_ge, op1=ALU.add,
                            accum_out=up[:, m, n:n + 1],
                        )
                    if u_out is not None:
                        nc.vector.tensor_reduce(
                            u_out[:, m:m + 1], up[:, m:m + 1, :],
                            mybir.AxisListType.X, ALU.add,
                        )

                def drain_one():
                    # one 2048-wide op; accum gives the u column directly
                    md = pending.pop(0)
                    acc = u_out[:, md:md + 1] if u_out is not None \
                        else up[:, md, 0:1]
                    nc.vector.tensor_scalar(
                        out=S_out[md // 2][:, md % 2, :],
                        in0=zdefs[md], scalar1=t_sc[:, md:md + 1],
                        scalar2=0.0, op0=ALU.is_ge, op1=ALU.add,
                        accum_out=acc,
                    )

                for m in range(MT):
                    psums = mains(m)
                    if l == 3:
                        # out = 2*p + (sign(b4) - rowsum(W4~)): ACT copy with
                        # scale then DVE add (both proven-fast op shapes)
                        mc = slice(m * 128, (m + 1) * 128)
                        for n in range(NCH):
                            ot = ou.tile([128, NF], mdt.float32, tag="ot",
                                         name=f"ot{m}_{n}")
                            nc.scalar.activation(ot, psums[n], ACTF.Copy,
                                                 bias=0.0, scale=2.0)
                            nc.vector.tensor_scalar_add(ot, ot, c4[:, m:m + 1])
                            nc.sync.dma_start(outT[mc, n * NF:(n + 1) * NF], ot)
                        continue
                    if m < ND2:
                        for n in range(NCH):
                            nc.scalar.copy(zdefs[m][:, n * NF:(n + 1) * NF],
                                           psums[n])
                        pending.append(m)
                    if m == TA:
                        tmm_group(0)
                    if m == ND2 - 1:
                        tmm_group(1)
                        t_combine()
                    if m >= ND2:
                        swrite_live(m, psums)
                        ndr = 1 if MT > 8 else 2
                        for _ in range(ndr):
                            if pending:
                                drain_one()
                    if halfar is not None and m == 9:
                        halfar(0)
                while pending:
                    drain_one()
                if halfar is not None:
                    halfar(1)

            def alloc_s(pool, H, nm):
                return [
                    pool.tile([128, 2, BL], FP8, tag="s", name=f"{nm}_{i}")
                    for i in range(H // 256)
                ]

            # layer 1
            S1 = alloc_s(sB, H1, "s1")
            u1 = stat.tile([128, H1 // 128], mdt.float32, tag="u1")
            layer(0, B0, W8_1, 0, S1, u1,
                  halfar=lambda h: ar_fire(1, u1, h))

            # layer 2 (W3 prep overlaps)
            W8_3 = [prep_w_pair(wA, 2, kp) for kp in range(H2 // 256)]
            S2 = alloc_s(sA, H2, "s2")
            u2 = stat.tile([128, H2 // 128], mdt.float32, tag="u2")
            layer(1, S1, W8_2, 1, S2, u2,
                  halfar=lambda h: ar_fire(2, u2, h))

            # W4 prep + rW4 = rowsum(sign(W4)) early so L4 never waits;
            # c4 = sb4 - rW4
            W8_4 = [prep_w_pair(wB, 3, kp) for kp in range(H3 // 256)]
            ones1 = stat.tile([128, 1], mdt.float16, tag="ones1")
            nc.vector.memset(ones1, 1.0)
            ptw = pz.tile([128, 512], mdt.float32, tag="pz", name="ptw")
            KT4 = H3 // 128
            for c in range(KT4):
                nc.tensor.matmul(
                    ptw[0:1, :], ones1, W8_4[c // 2][:, c % 2, :],
                    start=(c == 0), stop=(c == KT4 - 1),
                    skip_group_check=True,
                )
            rwsb = stat.tile([128, 512], mdt.float32, tag="rwsb")
            nc.vector.tensor_copy(out=rwsb[0:1, :], in_=ptw[0:1, :])
            nc.gpsimd.dma_start(rwscr[0, :], rwsb[0:1, :])
            rw4 = stat.tile([128, C // 128], mdt.float32, tag="rw4")
            nc.gpsimd.dma_start(
                rw4, rwscr.rearrange("o (m p) -> p (o m)", p=128),
            )
            c4 = stat.tile([128, C // 128], mdt.float32, tag="c4")
            nc.vector.tensor_tensor(out=c4, in0=sb4, in1=rw4, op=ALU.subtract)

            # layer 3
            S3 = alloc_s(sB, H3, "s3")
            layer(2, S2, W8_3, 2, S3, None)

            # layer 4 (no BN): out = 2*p - rW4 + sign(b4)
            layer(3, S3, W8_4, None, None, None)

    nc.compile()
    return nc


def _get_module():
    if "nc" not in _CACHE:
        _CACHE["nc"] = _build_module()
    return _CACHE["nc"]


def _reference_fallback(x, W1, b1, g1, be1, W2, b2, g2, be2, W3, b3, g3, be3, W4, b4):
    """Exact numpy clone of the reference for non-trivial gamma/beta inputs."""
    EPS = 1e-5

    def binarize(v):
        return np.where(v >= 0, 1.0, -1.0).astype(np.float32)

    def bin_linear(a, W, b):
        return binarize(a) @ binarize(W).T + binarize(b)

    def bn(z, g, be):
        m = z.mean(axis=0)
        v = z.var(axis=0)
        return (z - m) / np.sqrt(v + EPS) * g + be

    h = np.clip(bn(bin_linear(x, W1, b1), g1, be1), -1.0, 1.0)
    h = np.clip(bn(bin_linear(h, W2, b2), g2, be2), -1.0, 1.0)
    h = np.clip(bn(bin_linear(h, W3, b3), g3, be3), -1.0, 1.0)
    return bin_linear(h, W4, b4).astype(np.float32)


def _topbyte(a):
    """fp32 array -> top byte (sign+exponent msbs) as fp8e5: sign-exact."""
    a = np.ascontiguousarray(np.asarray(a, dtype=np.float32))
    return (a.view(np.uint32) >> 24).astype(np.uint8).view(ml_dtypes.float8_e5m2)


def make_in_maps(inputs):
    x8 = _topbyte(inputs["x"])
    common = {
        "w1t": np.ascontiguousarray(_topbyte(inputs["W1"]).T),
        "w2t": np.ascontiguousarray(_topbyte(inputs["W2"]).T),
        "w3t": np.ascontiguousarray(_topbyte(inputs["W3"]).T),
        "w4t": np.ascontiguousarray(_topbyte(inputs["W4"]).T),
        "b4": np.asarray(inputs["b4"], dtype=np.float32).reshape(C, 1),
    }
    in_maps = []
    for c in range(NCORES):
        m = dict(common)
        m["xT"] = np.ascontiguousarray(x8[c * BL:(c + 1) * BL, :].T)
        in_maps.append(m)
    return in_maps


def gather_output(results):
    out = np.empty((B, C), dtype=np.float32)
    for c in range(NCORES):
        out[c * BL:(c + 1) * BL, :] = results[c]["outT"].T.astype(np.float32)
    return out


def kernel(**inputs):
    # BN gamma/beta must be trivial for the sign-reduction; spec fills guarantee
    # this (g=ones, be=zeros).  Anything else falls back to exact host compute.
    for gk, bek in (("g1", "be1"), ("g2", "be2"), ("g3", "be3")):
        if not (np.all(np.asarray(inputs[gk]) == 1.0)
                and np.all(np.asarray(inputs[bek]) == 0.0)):
            return _reference_fallback(**{
                k: np.asarray(v, dtype=np.float32) for k, v in inputs.items()
            })

    from concourse.bass_utils import run_bass_kernel_spmd

    nc = _get_module()
    in_maps = make_in_maps(inputs)
    res = run_bass_kernel_spmd(nc, in_maps, list(range(NCORES)))
    return gather_output(res.results)


if __name__ == "__main__":
    nc = _get_module()
    print("module built OK")


# revision 16
# speedup vs baseline: 1.1417x; 1.0954x over previous
"""Trainium2 Bass kernel for nn_NeuralNetworkSimplified (binarized 4-layer MLP + BN).

Math
----
reference computes, per hidden layer l (gamma=1, beta=0, biases b_l arbitrary):
    z = sign(a) @ sign(W).T + sign(b)
    h = clip(batchnorm_train(z), -1, 1)
and the next layer only consumes sign(h).  Since batchnorm's rsqrt(var+eps) > 0
and gamma=1/beta=0, sign(h) = sign(z - mean_batch(z)); the sign(b) bias shifts
z and its mean equally, so it cancels.  The network reduces to exact integer
arithmetic.  On device we use an asymmetric encoding:
    W~ = sign(W) in {-1, +1}   (fp8, exact)
    B  = 1{a >= 0} in {0, 1}   (fp8, exact)
    p[m,n] = sum_k W~[k,m] * B[k,n]        (psum, exact ints)
    true Z = 2p - rowsum(W~)[m]; the rowsum term is constant per feature m, so
    it cancels in the batch-mean comparison:
        sign(Z - colmean(Z)) = 1{p >= pmean},  pmean = (W~ @ u) / 16384,
    where u[k] = global colsum of B (AllReduce of per-core counts).
The threshold pmean is computed on the PE with a tiny 2-column stationary
[u - 8192 (fp16 exact, |.| <= 2048), 8192] against the already-resident W~
tiles (moving operand), in 32-aligned column groups of one PSUM bank.
The final layer outputs 2*p4 - rowsum(W4~) + sign(b4).

Host prep is layout-only: inputs ship as the TOP BYTE of each fp32 value
(a pure bit-slice), reinterpreted as fp8e5 -- the sign is preserved exactly,
halving input DMA; all FLOPs including every sign() run on device.

Scheduling (v4):
 - per hidden layer, blocks m=0..4 spill psum->SBUF(fp16, exact small counts)
   via ACT so the PE never waits on the batch-mean AllReduce; the threshold
   matmuls run as two accumulation groups (first K-half after m=3's mains,
   second after m=4's), giving each 4KB AllReduce (~20-33us latency) a ~35us
   grace window.
 - u AllReduce halves fire as soon as their feature columns complete (after
   the m=9 drain); the dig (threshold stationary) builds are EMITTED at the
   consuming layer so they never sit at an engine queue head blocking
   unrelated work on the AllReduce.
 - S-writes and drains always use the DVE accum_out path (the plain
   tensor_scalar path measures ~4x slower); drains are single 2048-wide ops
   producing their u column directly.
 - the t-row shuffle runs its two row-chains on sync and gpsimd in parallel.
"""

import numpy as np
import ml_dtypes

B, D, H1, H2, H3, C = 16384, 3072, 2048, 2048, 1024, 512
NCORES = 8
BL = B // NCORES          # 2048 rows per core
NF = 512                  # batch free-dim chunk (one psum bank)
NCH = BL // NF            # 4 chunks
LAYERS = [(D, H1), (H1, H2), (H2, H3), (H3, C)]
UDIMS = [D, H1, H2]       # length of u vector feeding each hidden layer's mean

_CACHE = {}


def _build_module():
    import concourse.bass as bass
    import concourse.mybir as mybir
    import concourse.tile as tile
    from concourse import bacc

    mdt = mybir.dt
    FP8 = mdt.float8e4
    RAW = mdt.float8e5          # byte-truncated fp32: sign-exact
    ALU = mybir.AluOpType
    ACTF = mybir.ActivationFunctionType
    DR = mybir.MatmulPerfMode.DoubleRow

    nc = bacc.Bacc(
        "TRN2",
        target_bir_lowering=False,
        debug=False,
        num_devices=NCORES,
    )

    xT = nc.dram_tensor("xT", [D, BL], RAW, kind="ExternalInput").ap()
    wT = [
        nc.dram_tensor(f"w{i + 1}t", [K, H], RAW, kind="ExternalInput").ap()
        for i, (K, H) in enumerate(LAYERS)
    ]
    b4 = nc.dram_tensor("b4", [C, 1], mdt.float32, kind="ExternalInput").ap()
    outT = nc.dram_tensor("outT", [C, BL], mdt.float32, kind="ExternalOutput").ap()

    cc_in = [
        [nc.dram_tensor(f"cc_in{i}_{h}", [128, U // 256], mdt.float32).ap()
         for h in range(2)]
        for i, U in enumerate(UDIMS)
    ]
    cc_out = [
        [nc.dram_tensor(f"cc_out{i}_{h}", [128, U // 256], mdt.float32,
                        addr_space="Shared").ap()
         for h in range(2)]
        for i, U in enumerate(UDIMS)
    ]
    # scratch for cross-partition shuffle of the threshold rows (t) per layer
    tscr = [
        nc.dram_tensor(f"tscr{i}", [2, H // 512, 512], mdt.float32).ap()
        for i, H in enumerate([H1, H2, H3])
    ]
    rwscr = nc.dram_tensor("rwscr", [1, C], mdt.float32).ap()
    ccw_in = nc.dram_tensor("ccw_in", [128, 1], mdt.float32).ap()
    ccw_out = nc.dram_tensor("ccw_out", [128, 1], mdt.float32, addr_space="Shared").ap()

    with tile.TileContext(nc, num_cores=NCORES) as tc:
        with (
            tc.tile_pool(name="raw", bufs=2) as raw,       # staging raw-byte tiles
            tc.tile_pool(name="sA", bufs=12) as sA,        # B0, S2 pair tiles
            tc.tile_pool(name="sB", bufs=8) as sB,         # S1, S3 pair tiles
            tc.tile_pool(name="wA", bufs=12) as wA,        # W1~, W3~ pair tiles
            tc.tile_pool(name="wB", bufs=8) as wB,         # W2~, W4~ pair tiles
            tc.tile_pool(name="stat", bufs=1) as stat,     # u/t/bias vectors
            tc.tile_pool(name="zd", bufs=6) as zd,         # deferred psum spills
            tc.tile_pool(name="ou", bufs=4) as ou,         # output staging
            tc.tile_pool(name="pz", bufs=8, space="PSUM") as pz,
        ):
            # ---- PE warm-up while the prologue runs on DMA/DVE ----
            warm = stat.tile([128, 128], FP8, tag="warm")
            nc.vector.memset(warm, 1.0)
            wps = pz.tile([128, 128], mdt.float32, tag="pz", name="warmps")
            for i in range(36):
                nc.tensor.matmul(wps, warm, warm, start=True, stop=True)

            # warm the collective stream early
            ccwt = stat.tile([128, 1], mdt.float32, tag="ccwt")
            nc.vector.memset(ccwt, 0.0)
            nc.gpsimd.dma_start(ccw_in, ccwt)
            nc.gpsimd.collective_compute(
                "AllReduce", ALU.add,
                replica_groups=[list(range(NCORES))],
                ins=[ccw_in], outs=[ccw_out],
            )

            # tiny positive bias for ACT Sign: breaks w==0 ties toward +1
            biap = stat.tile([128, 1], mdt.float32, tag="biap")
            nc.vector.memset(biap, 1e-30)

            # ---- sign(b4) as +-1 per-partition vector, [128, C//128] ----
            b4_sb = stat.tile([128, C // 128], mdt.float32, tag="rawb")
            nc.sync.dma_start(b4_sb, b4.rearrange("(o p) q -> p (o q)", p=128))
            sb4 = stat.tile([128, C // 128], mdt.float32, tag="sb4")
            nc.gpsimd.tensor_scalar(
                out=sb4, in0=b4_sb, scalar1=0.0, scalar2=2.0,
                op0=ALU.is_ge, op1=ALU.mult,
            )
            nc.gpsimd.tensor_scalar_add(sb4, sb4, -1.0)

            # ---- binarize helpers ----
            def prep_w_pair(pool, idx, kp):
                """raw-byte dram pair -> fp8 +-1 pair tile (sign, exact)."""
                K, H = LAYERS[idx]
                w8 = pool.tile([128, 2, H], FP8, tag="w", name=f"w{idx}_{kp}")
                for h in range(2):
                    rt = raw.tile([128, H], RAW, tag="raww",
                                  name=f"rw{idx}_{kp}_{h}")
                    nc.sync.dma_start(
                        rt,
                        wT[idx][(2 * kp + h) * 128:(2 * kp + h + 1) * 128, :],
                    )
                    nc.scalar.activation(w8[:, h, :], rt,
                                         ACTF.Sign, bias=biap, scale=1.0)
                return w8

            # ---- u AllReduce fire (dig build deferred to the consumer) ----
            def ar_fire(idx, u_sb, half):
                HC = UDIMS[idx] // 256
                sl = slice(half * HC, (half + 1) * HC)
                nc.gpsimd.dma_start(cc_in[idx][half], u_sb[:, sl])
                nc.gpsimd.collective_compute(
                    "AllReduce",
                    ALU.add,
                    replica_groups=[list(range(NCORES))],
                    ins=[cc_in[idx][half]],
                    outs=[cc_out[idx][half]],
                )

            def dig_build(idx, half):
                """AR result -> dig stationary [128, HC, 2] (fp16 exact)."""
                HC = UDIMS[idx] // 256
                ug = stat.tile([128, HC], mdt.float32, tag=f"ug{idx}_{half}")
                nc.gpsimd.dma_start(ug, cc_out[idx][half])
                dig = stat.tile([128, HC, 2], mdt.float16, tag=f"dig{idx}_{half}")
                nc.vector.memset(dig[:, :, 1:2], 8192.0)
                nc.vector.tensor_scalar_add(dig[:, :, 0:1], ug.unsqueeze(2),
                                            -8192.0)
                return dig

            # ---- x prep: B0 = 1{x>=0} (fp8 {0,1}) + u0 counts fused ----
            B0 = []
            u0 = stat.tile([128, D // 128], mdt.float32, tag="u0")
            W8_1 = [prep_w_pair(wA, 0, 0)]
            for kp in range(D // 256):
                s8 = sA.tile([128, 2, BL], FP8, tag="s", name=f"s0_{kp}")
                for h in range(2):
                    rt = raw.tile([128, BL], RAW, tag="rawx",
                                  name=f"rx{kp}_{h}")
                    nc.sync.dma_start(
                        rt,
                        xT[(2 * kp + h) * 128:(2 * kp + h + 1) * 128, :],
                    )
                    nc.vector.tensor_scalar(
                        out=s8[:, h, :], in0=rt, scalar1=0.0,
                        scalar2=0.0, op0=ALU.is_ge, op1=ALU.add,
                        accum_out=u0[:, 2 * kp + h:2 * kp + h + 1],
                    )
                B0.append(s8)
                # x leads W1 2:1 so u0 (and its AllReduce) finish early
                if kp % 2 == 1 and kp // 2 + 1 < D // 512:
                    W8_1.append(prep_w_pair(wA, 0, kp // 2 + 1))
                if kp == D // 512 - 1:
                    ar_fire(0, u0, 0)
            ar_fire(0, u0, 1)
            for kp in range(D // 512, D // 256):
                W8_1.append(prep_w_pair(wA, 0, kp))

            W8_2 = [prep_w_pair(wB, 1, kp) for kp in range(H1 // 256)]

            # ---- one layer ----
            def layer(l, A8, W8, uidx, S_out, u_out, halfar=None):
                K, H = LAYERS[l]
                KT, MT = K // 128, H // 128
                NS = H // 512
                # L2 defers one more block: its t-b AllReduce measures ~42us
                # start-to-finish, needing ~41us of grace (6 blocks)
                ND2, TA = (6, 3) if l == 1 else (5, 3)
                t_sc = stat.tile([128, MT], mdt.float32, tag=f"tsc{l}",
                                 name=f"tsc{l}") if l < 3 else None
                up = stat.tile([128, MT, NCH], mdt.float32, tag=f"up{l}",
                               name=f"up{l}") if l < 3 else None
                zdefs = [zd.tile([128, BL], mdt.float16, tag="zd",
                                 name=f"zd{l}_{m}") for m in range(ND2)] \
                    if l < 3 else None
                ptl = [None]
                pending = []

                def mains(m):
                    mc = slice(m * 128, (m + 1) * 128)
                    psums = [
                        pz.tile([128, NF], mdt.float32, tag="pz",
                                name=f"pz{l}_{m}_{n}")
                        for n in range(NCH)
                    ]
                    for kp in range(KT // 2):
                        st, sp = kp == 0, kp == KT // 2 - 1
                        wsl = W8[kp][:, :, mc]
                        for n in range(NCH):
                            nc.tensor.matmul(
                                psums[n], wsl, A8[kp][:, :, n * NF:(n + 1) * NF],
                                start=st, stop=sp, perf_mode=DR,
                            )
                    return psums

                def tmm_group(grp):
                    if grp == 0:
                        ptl[0] = pz.tile([128, 512], mdt.float32, tag="pz",
                                         name=f"pt{l}")
                    dg = dig_build(uidx, grp)
                    c0, c1 = (0, KT // 2) if grp == 0 else (KT // 2, KT)
                    for c in range(c0, c1):
                        dc = c - c0
                        for s in range(NS):
                            nc.tensor.matmul(
                                ptl[0][32 * s:32 * s + 2, :],
                                dg[:, dc, :],
                                W8[c // 2][:, c % 2, 512 * s:512 * s + 512],
                                start=(c == 0), stop=(c == KT - 1),
                                skip_group_check=True,
                                tile_position=(0, 32 * s),
                            )

                def t_combine():
                    # psum rows -> sbuf -> [128, MT]; r=0 on sync, r=1 on
                    # gpsimd so the two shuffle chains run in parallel
                    tsb = stat.tile([128, 512], mdt.float32, tag="tsb")
                    nc.vector.tensor_copy(out=tsb, in_=ptl[0])
                    t2a = stat.tile([128, MT], mdt.float32, tag=f"t2a{l}")
                    t2b = stat.tile([128, MT], mdt.float32, tag=f"t2b{l}")
                    for r, t2x in enumerate((t2a, t2b)):
                        eng = nc.sync if r == 0 else nc.gpsimd
                        for s in range(NS):
                            eng.dma_start(
                                tscr[l][r, s, :],
                                tsb[32 * s + r:32 * s + r + 1, :],
                            )
                        eng.dma_start(
                            t2x,
                            tscr[l][r:r + 1]
                            .rearrange("o g (q p) -> p (o g q)", p=128),
                        )
                    nc.vector.tensor_tensor(
                        out=t_sc, in0=t2a, in1=t2b, op=ALU.add,
                    )
                    nc.vector.tensor_scalar_mul(t_sc, t_sc, 1.0 / B)

                def swrite_live(m, psums):
                    # S_out = 1{p >= t}; per-chunk u partials via accum_out
                    # (always accum: the plain path is ~4x slower)
                    for n in range(NCH):
                        nc.vector.tensor_scalar(
                            out=S_out[m // 2][:, m % 2, n * NF:(n + 1) * NF],
                            in0=psums[n], scalar1=t_sc[:, m:m + 1],
                            scalar2=0.0, op0=ALU.is_ge, op1=ALU.add,
                            accum_out=up[:, m, n:n + 1],
                        )
                    if u_out is not None:
                        nc.vector.tensor_reduce(
                            u_out[:, m:m + 1], up[:, m:m + 1, :],
                            mybir.AxisListType.X, ALU.add,
                        )

                def drain_one():
                    # one 2048-wide op; accum gives the u column directly
                    md = pending.pop(0)
                    acc = u_out[:, md:md + 1] if u_out is not None \
                        else up[:, md, 0:1]
                    nc.vector.tensor_scalar(
                        out=S_out[md // 2][:, md % 2, :],
                        in0=zdefs[md], scalar1=t_sc[:, md:md + 1],
                        scalar2=0.0, op0=ALU.is_ge, op1=ALU.add,
                        accum_out=acc,
                    )

                for m in range(MT):
                    psums = mains(m)
                    if l == 3:
                        # out = 2*p + (sign(b4) - rowsum(W4~)): ACT copy with
                        # scale then DVE add (both proven-fast op shapes)
                        mc = slice(m * 128, (m + 1) * 128)
                        for n in range(NCH):
                            ot = ou.tile([128, NF], mdt.float32, tag="ot",
                                         name=f"ot{m}_{n}")
                            nc.scalar.activation(ot, psums[n], ACTF.Copy,
                                                 bias=0.0, scale=2.0)
                            nc.vector.tensor_scalar_add(ot, ot, c4[:, m:m + 1])
                            nc.sync.dma_start(outT[mc, n * NF:(n + 1) * NF], ot)
                        continue
                    if m < ND2:
                        for n in range(NCH):
                            nc.scalar.copy(zdefs[m][:, n * NF:(n + 1) * NF],
                                           psums[n])
                        pending.append(m)
                    if m == TA:
                        tmm_group(0)
                    if m == ND2 - 1:
                        tmm_group(1)
                        t_combine()
                    if m >= ND2:
                        swrite_live(m, psums)
                        ndr = 1 if MT > 8 else 2
                        for _ in range(ndr):
                            if pending:
                                drain_one()
                    if halfar is not None and m == 2 * ND2 - 1:
                        halfar(0)
                while pending:
                    drain_one()
                if halfar is not None:
                    halfar(1)

            def alloc_s(pool, H, nm):
                return [
                    pool.tile([128, 2, BL], FP8, tag="s", name=f"{nm}_{i}")
                    for i in range(H // 256)
                ]

            # layer 1
            S1 = alloc_s(sB, H1, "s1")
            u1 = stat.tile([128, H1 // 128], mdt.float32, tag="u1")
            layer(0, B0, W8_1, 0, S1, u1,
                  halfar=lambda h: ar_fire(1, u1, h))

            # layer 2 (W3 prep overlaps)
            W8_3 = [prep_w_pair(wA, 2, kp) for kp in range(H2 // 256)]
            S2 = alloc_s(sA, H2, "s2")
            u2 = stat.tile([128, H2 // 128], mdt.float32, tag="u2")
            layer(1, S1, W8_2, 1, S2, u2,
                  halfar=lambda h: ar_fire(2, u2, h))

            # W4 prep + rW4 = rowsum(sign(W4)) early so L4 never waits;
            # c4 = sb4 - rW4
            W8_4 = [prep_w_pair(wB, 3, kp) for kp in range(H3 // 256)]
            ones1 = stat.tile([128, 1], mdt.float16, tag="ones1")
            nc.vector.memset(ones1, 1.0)
            ptw = pz.tile([128, 512], mdt.float32, tag="pz", name="ptw")
            KT4 = H3 // 128
            for c in range(KT4):
                nc.tensor.matmul(
                    ptw[0:1, :], ones1, W8_4[c // 2][:, c % 2, :],
                    start=(c == 0), stop=(c == KT4 - 1),
                    skip_group_check=True,
                )
            rwsb = stat.tile([128, 512], mdt.float32, tag="rwsb")
            nc.vector.tensor_copy(out=rwsb[0:1, :], in_=ptw[0:1, :])
            nc.gpsimd.dma_start(rwscr[0, :], rwsb[0:1, :])
            rw4 = stat.tile([128, C // 128], mdt.float32, tag="rw4")
            nc.gpsimd.dma_start(
                rw4, rwscr.rearrange("o (m p) -> p (o m)", p=128),
            )
            c4 = stat.tile([128, C // 128], mdt.float32, tag="c4")
            nc.vector.tensor_tensor(out=c4, in0=sb4, in1=rw4, op=ALU.subtract)

            # layer 3
            S3 = alloc_s(sB, H3, "s3")
            layer(2, S2, W8_3, 2, S3, None)

            # layer 4 (no BN): out = 2*p - rW4 + sign(b4)
            layer(3, S3, W8_4, None, None, None)

    nc.compile()
    return nc


def _get_module():
    if "nc" not in _CACHE:
        _CACHE["nc"] = _build_module()
    return _CACHE["nc"]


def _reference_fallback(x, W1, b1, g1, be1, W2, b2, g2, be2, W3, b3, g3, be3, W4, b4):
    """Exact numpy clone of the reference for non-trivial gamma/beta inputs."""
    EPS = 1e-5

    def binarize(v):
        return np.where(v >= 0, 1.0, -1.0).astype(np.float32)

    def bin_linear(a, W, b):
        return binarize(a) @ binarize(W).T + binarize(b)

    def bn(z, g, be):
        m = z.mean(axis=0)
        v = z.var(axis=0)
        return (z - m) / np.sqrt(v + EPS) * g + be

    h = np.clip(bn(bin_linear(x, W1, b1), g1, be1), -1.0, 1.0)
    h = np.clip(bn(bin_linear(h, W2, b2), g2, be2), -1.0, 1.0)
    h = np.clip(bn(bin_linear(h, W3, b3), g3, be3), -1.0, 1.0)
    return bin_linear(h, W4, b4).astype(np.float32)


def _topbyte(a):
    """fp32 array -> top byte (sign+exponent msbs) as fp8e5: sign-exact."""
    a = np.ascontiguousarray(np.asarray(a, dtype=np.float32))
    return (a.view(np.uint32) >> 24).astype(np.uint8).view(ml_dtypes.float8_e5m2)


def make_in_maps(inputs):
    x8 = _topbyte(inputs["x"])
    common = {
        "w1t": np.ascontiguousarray(_topbyte(inputs["W1"]).T),
        "w2t": np.ascontiguousarray(_topbyte(inputs["W2"]).T),
        "w3t": np.ascontiguousarray(_topbyte(inputs["W3"]).T),
        "w4t": np.ascontiguousarray(_topbyte(inputs["W4"]).T),
        "b4": np.asarray(inputs["b4"], dtype=np.float32).reshape(C, 1),
    }
    in_maps = []
    for c in range(NCORES):
        m = dict(common)
        m["xT"] = np.ascontiguousarray(x8[c * BL:(c + 1) * BL, :].T)
        in_maps.append(m)
    return in_maps


def gather_output(results):
    out = np.empty((B, C), dtype=np.float32)
    for c in range(NCORES):
        out[c * BL:(c + 1) * BL, :] = results[c]["outT"].T.astype(np.float32)
    return out


def kernel(**inputs):
    # BN gamma/beta must be trivial for the sign-reduction; spec fills guarantee
    # this (g=ones, be=zeros).  Anything else falls back to exact host compute.
    for gk, bek in (("g1", "be1"), ("g2", "be2"), ("g3", "be3")):
        if not (np.all(np.asarray(inputs[gk]) == 1.0)
                and np.all(np.asarray(inputs[bek]) == 0.0)):
            return _reference_fallback(**{
                k: np.asarray(v, dtype=np.float32) for k, v in inputs.items()
            })

    from concourse.bass_utils import run_bass_kernel_spmd

    nc = _get_module()
    in_maps = make_in_maps(inputs)
    res = run_bass_kernel_spmd(nc, in_maps, list(range(NCORES)))
    return gather_output(res.results)


if __name__ == "__main__":
    nc = _get_module()
    print("module built OK")
